# revision 1
# baseline (speedup 1.0000x reference)
"""Trainium2 Bass kernel for nn_DenoisingConditionDecoder.

Per-core computation (data-parallel over batch, 1 batch element per core):
  gate  = sigmoid([nx, cond] @ W_gate + b_gate)
  fused = gate*nx + (1-gate)*cond
  attn  = softmax(fused @ X^T / sqrt(D)) @ X
  q     = LN(fused + attn) * g1 + be1
  ff    = gelu(q @ W1 + b1) @ W2 + b2
  out   = LN(q + ff) * g2 + be2

Layout strategy: activations feeding matmul contractions live transposed
([feat, seq], "T" layout) so the contraction dim sits on partitions;
softmax-normalize / layernorm / residuals run in natural [seq, feat]
layout.  Layout changes ride the DMA engines: bf16 copies are written to
DRAM scratch (SWDGE) and read back with the HWDGE xbar transpose on the
ACT ring, keeping PE/ACT/DVE compute free of transpose traffic and the
sync ring free of xbar-mode switches.

Attention uses a scores-transposed formulation: scoresT[k, q] tiles come
out of the PE with k on partitions, exp runs on ACT into bf16 tiles which
are directly the lhsT (stationary) operand of attn_out = attn @ X.
Softmax row sums accumulate as ones-lhsT [1, QB] matmuls after the scores
chains; a tiny PE transpose gives per-partition reciprocals.  FFN2 uses
the same trick: lhsT = ff1T tiles, rhs = W2 rows, producing ff2 directly
in natural layout for the residual.

Schedule: gate/fusion for all blocks runs first (retiring the condT/nxmcT
buffers, whose slots are then recycled as double-buffered exp blocks);
then a software pipeline over 512-query blocks emits FFN(qb-1) between
scores(qb) and attn(qb) so TensorE never waits on the LN1 -> q^T DMA
round-trip or the exp ACT latency.
"""

import math
import numpy as np

import concourse.bass as bass
import concourse.tile as tile
from concourse import bacc, mybir
from concourse.bass_utils import run_bass_kernel_spmd
from concourse.masks import make_identity

B, S, D = 8, 2048, 512
H = 2 * D
P = 128
NT = S // P   # 16 seq tiles
DT = D // P   # 4 feature tiles
HT = H // P   # 8 hidden tiles
QB = 512      # q-block
NQB = S // QB # 4
JB = QB // P  # 4 q-subtiles per block
LN_EPS = 1e-5
SCALE = 1.0 / math.sqrt(D)

F32 = mybir.dt.float32
BF16 = mybir.dt.bfloat16
AF = mybir.ActivationFunctionType
ALU = mybir.AluOpType

N_CORES = 8

_cache = {}


def _build(gelu_func=None, affine1=True, affine2=True, bias2=True):
    gelu_func = AF.Gelu if gelu_func is None else gelu_func
    nc = bacc.Bacc("TRN2", target_bir_lowering=False, debug=False,
                   num_devices=N_CORES)

    dr = {}
    for nm, shp in [("nx", [S, D]), ("x", [S, D]), ("cond", [S, D]),
                    ("wg", [H, D]), ("bg", [D]), ("w1", [D, H]), ("b1", [H]),
                    ("w2", [H, D]), ("b2", [D]), ("g1", [D]), ("be1", [D]),
                    ("g2", [D]), ("be2", [D])]:
        dr[nm] = nc.dram_tensor(nm, shp, F32, kind="ExternalInput")
    dr["out"] = nc.dram_tensor("out", [S, D], F32, kind="ExternalOutput")

    with tile.TileContext(nc) as tc:
        _body(nc, tc, dr, gelu_func, affine1, affine2, bias2)
    nc.compile()
    return nc


def _body(nc, tc, dr, gelu_func, affine1, affine2, bias2):
    from contextlib import ExitStack

    # DRAM scratch for DMA-based transposes (bf16)
    scr_x = nc.dram_tensor("scr_x", [S, D], BF16)
    scr_fT = nc.dram_tensor("scr_fT", [D, S], BF16)
    scr_q = nc.dram_tensor("scr_q", [S, D], BF16)

    ctx = ExitStack()
    with ctx:
        # ---------------- pools ----------------
        const = ctx.enter_context(tc.tile_pool(name="const", bufs=1))
        p_stg = ctx.enter_context(tc.tile_pool(name="stg", bufs=4))
        p_w = ctx.enter_context(tc.tile_pool(name="w", bufs=1))
        p_big = ctx.enter_context(tc.tile_pool(name="big", bufs=4))
        p_seq = ctx.enter_context(tc.tile_pool(name="seq", bufs=16))
        p_qb = ctx.enter_context(tc.tile_pool(name="qbp", bufs=8))
        p_cst = ctx.enter_context(tc.tile_pool(name="cst", bufs=6))
        p_sm = ctx.enter_context(tc.tile_pool(name="sm", bufs=8))
        p_xh = ctx.enter_context(tc.tile_pool(name="xh", bufs=4))

        ps_mm = ctx.enter_context(tc.tile_pool(name="psmm", bufs=4,
                                               space="PSUM"))
        ps_tr = ctx.enter_context(tc.tile_pool(name="pstr", bufs=4,
                                               space="PSUM"))

        # ---------------- constants ----------------
        ident_f = const.tile([P, P], F32, tag="idf")
        make_identity(nc, ident_f)
        ident_b = const.tile([P, P], BF16, tag="idb")
        make_identity(nc, ident_b)
        ones_b = const.tile([P, 1], BF16, tag="ones")
        nc.vector.memset(ones_b, 1.0)
        eps_t = const.tile([P, 1], F32, tag="eps")
        nc.vector.memset(eps_t, LN_EPS)

        def bcast_vec(dram, tag):
            t = const.tile([P, D], F32, tag=tag)
            a = dram.ap()
            src = bass.AP(tensor=a.tensor, offset=a.offset,
                          ap=[[0, P]] + list(a.ap))
            nc.sync.dma_start(out=t, in_=src)
            return t

        g1b = bcast_vec(dr["g1"], "g1") if affine1 else None
        be1b = bcast_vec(dr["be1"], "be1") if affine1 else None
        g2b = bcast_vec(dr["g2"], "g2") if affine2 else None
        be2b = bcast_vec(dr["be2"], "be2") if affine2 else None
        b2b = bcast_vec(dr["b2"], "b2v") if bias2 else None

        def part_vec(dram, n, tag):
            ts = []
            for m in range(n):
                t = p_sm.tile([P, 1], F32, tag=tag, bufs=n, name=f"{tag}{m}")
                nc.sync.dma_start(
                    out=t, in_=dram.ap()[m * P:(m + 1) * P].unsqueeze(1))
                ts.append(t)
            return ts

        bg_sb = part_vec(dr["bg"], DT, "bg")
        b1_sb = part_vec(dr["b1"], HT, "b1")

        # ---------------- weights (cast f32 -> bf16 during DMA) ----------
        def load_w(dram, n, cols, tag, pref):
            ts = []
            for k in range(n):
                t = p_w.tile([P, cols], BF16, tag=tag, bufs=n,
                             name=f"{pref}{k}")
                nc.gpsimd.dma_start(out=t, in_=dram.ap()[k * P:(k + 1) * P, :])
                ts.append(t)
            return ts

        wg_b = load_w(dr["wg"], HT, D, "wg", "wg")
        # gate uses combined=[nx,cond]; rewrite with nxmc=(nx-cond):
        #   logits = nxmc @ Wg_top + cond @ (Wg_top + Wg_bot)
        for k in range(DT):
            nc.vector.tensor_add(wg_b[k + DT], wg_b[k + DT], wg_b[k])

        # ------- stage 1: load (batched), cast, scratch, transpose-load ---
        XN = [p_seq.tile([P, D], BF16, tag="XN", bufs=NT,
                         name=f"XN{j}") for j in range(NT)]
        condT = [p_big.tile([P, S], BF16, tag="condT", bufs=DT,
                            name=f"condT{j}") for j in range(DT)]
        nxmcT = [p_big.tile([P, S], BF16, tag="nxmcT", bufs=DT,
                            name=f"nxmcT{j}") for j in range(DT)]
        XT = [p_big.tile([P, S], BF16, tag="XT", bufs=DT,
                         name=f"XT{j}") for j in range(DT)]

        for qc in range(NQB):
            for a in range(2):  # two 256-row chunks per q-block
                rows = slice((qc * 2 + a) * 2 * P, (qc * 2 + a + 1) * 2 * P)
                nx_s = p_stg.tile([P, 2, D], F32, tag="stg", name=f"nxs{qc}_{a}")
                nc.sync.dma_start(
                    out=nx_s, in_=dr["nx"].ap()[rows, :].rearrange(
                        "(t p) d -> p t d", p=P))
                cond_s = p_stg.tile([P, 2, D], F32, tag="stg",
                                    name=f"cds{qc}_{a}")
                nc.sync.dma_start(
                    out=cond_s, in_=dr["cond"].ap()[rows, :].rearrange(
                        "(t p) d -> p t d", p=P))
                x_s = p_stg.tile([P, 2, D], F32, tag="stg", name=f"xs{qc}_{a}")
                nc.sync.dma_start(
                    out=x_s, in_=dr["x"].ap()[rows, :].rearrange(
                        "(t p) d -> p t d", p=P))
                for h in range(2):
                    i = (qc * 2 + a) * 2 + h
                    row = slice(i * P, (i + 1) * P)
                    nc.vector.tensor_copy(out=XN[i], in_=x_s[:, h, :])
                    nc.sync.dma_start(out=scr_x.ap()[row, :], in_=XN[i])
                    cb = p_cst.tile([P, D], BF16, tag="cst", name=f"cb{i}")
                    nc.vector.tensor_copy(out=cb, in_=cond_s[:, h, :])
                    sb = p_cst.tile([P, D], BF16, tag="cst", name=f"sb{i}")
                    nc.vector.tensor_sub(sb, nx_s[:, h, :], cond_s[:, h, :])
                    for j in range(DT):
                        col = slice(j * P, (j + 1) * P)
                        ptc = ps_tr.tile([P, P], BF16, tag="tr",
                                         name=f"ptc{i}_{j}")
                        nc.tensor.transpose(ptc, cb[:, col], ident_b)
                        nc.any.tensor_copy(out=condT[j][:, row], in_=ptc)
                        ptn = ps_tr.tile([P, P], BF16, tag="tr",
                                         name=f"ptn{i}_{j}")
                        nc.tensor.transpose(ptn, sb[:, col], ident_b)
                        nc.any.tensor_copy(out=nxmcT[j][:, row], in_=ptn)

        for j in range(DT):
            col = slice(j * P, (j + 1) * P)
            nc.sync.dma_start(out=XT[j], in_=scr_x.ap()[:, col],
                              transpose=True)

        # ------- stage 2: gate matmul + fusion for ALL blocks -------------
        fT = [p_big.tile([P, S], BF16, tag="fT", bufs=DT,
                         name=f"fT{j}") for j in range(DT)]
        for qc in range(NQB):
            qs = slice(qc * QB, (qc + 1) * QB)
            for m in range(DT):
                mcol = slice(m * P, (m + 1) * P)
                ps = ps_mm.tile([P, QB], F32, tag="mm", name=f"psg{qc}_{m}")
                for k in range(HT):
                    src = nxmcT[k] if k < DT else condT[k - DT]
                    nc.tensor.matmul(ps, wg_b[k][:, mcol], src[:, qs],
                                     start=(k == 0), stop=(k == HT - 1))
                gt = p_cst.tile([P, QB], BF16, tag="cst", name=f"gt{qc}_{m}")
                nc.scalar.activation(gt, ps, AF.Sigmoid, bias=bg_sb[m])
                # fusedT = cond + gate * (nx - cond)
                nc.vector.tensor_mul(gt, gt, nxmcT[m][:, qs])
                nc.vector.tensor_add(fT[m][:, qs], gt, condT[m][:, qs])
                nc.sync.dma_start(out=scr_fT.ap()[mcol, qs],
                                  in_=fT[m][:, qs])
        w1_b = load_w(dr["w1"], DT, H, "w1", "w1")
        w2_b = load_w(dr["w2"], HT, D, "w2", "w2")
        fN = [p_seq.tile([P, D], BF16, tag="fN", bufs=NT, name=f"fN{qi}")
              for qi in range(NT)]
        for qi in range(NT):
            nc.sync.dma_start(out=fN[qi],
                                in_=scr_fT.ap()[:, qi * P:(qi + 1) * P],
                                transpose=True)

        # ------- per-q-block software pipeline ----------------------------
        state = {}

        def emit_scores(qb):
            qs = slice(qb * QB, (qb + 1) * QB)
            # exp blocks recycle condT (even qb) / nxmcT (odd qb) slots
            etag = "condT" if qb % 2 == 0 else "nxmcT"
            eblk = [p_big.tile([P, S], BF16, tag=etag, bufs=DT,
                               name=f"eblk{qb}_{b}") for b in range(DT)]
            for kt in range(NT):
                ps = ps_mm.tile([P, QB], F32, tag="mm", name=f"pss{qb}_{kt}")
                for dj in range(DT):
                    nc.tensor.matmul(ps, XT[dj][:, kt * P:(kt + 1) * P],
                                     fT[dj][:, qs],
                                     start=(dj == 0), stop=(dj == DT - 1))
                esl = eblk[kt // DT][:, (kt % DT) * QB:(kt % DT + 1) * QB]
                nc.scalar.activation(esl, ps, AF.Exp, scale=SCALE)
            prs = ps_tr.tile([1, QB], F32, tag="tr", name=f"prs{qb}")
            for kt in range(NT):
                esl = eblk[kt // DT][:, (kt % DT) * QB:(kt % DT + 1) * QB]
                nc.tensor.matmul(prs, ones_b, esl,
                                 start=(kt == 0), stop=(kt == NT - 1))
            rs_sb = p_sm.tile([1, QB], F32, tag="rs", bufs=4,
                              name=f"rssb{qb}")
            nc.vector.tensor_copy(out=rs_sb, in_=prs)
            state[qb] = (eblk, rs_sb)

        def emit_attn_ln1_qt(qb):
            eblk, rs_sb = state[qb]
            qs = slice(qb * QB, (qb + 1) * QB)
            qNb = []
            for j in range(JB):
                qi = qb * JB + j
                pa = ps_mm.tile([P, D], F32, tag="mm", name=f"psa{qi}")
                for kt in range(NT):
                    lhs = eblk[kt // DT][:, (kt % DT) * QB + j * P:
                                         (kt % DT) * QB + (j + 1) * P]
                    nc.tensor.matmul(pa, lhs, XN[kt],
                                     start=(kt == 0), stop=(kt == NT - 1))
                prt = ps_tr.tile([P, 1], F32, tag="tr", name=f"prt{qi}")
                nc.tensor.transpose(prt, rs_sb[0:1, j * P:(j + 1) * P],
                                    ident_f[0:1, 0:1])
                rec = p_sm.tile([P, 1], F32, tag="rec", name=f"rec{qi}")
                nc.vector.reciprocal(rec, prt)
                qn = p_qb.tile([P, D], F32, tag="qN", bufs=8, name=f"qN{qi}")
                # r1 = attn_out/rowsum + fused   (LN1 runs in place)
                nc.vector.scalar_tensor_tensor(
                    qn, pa, rec, fN[qi], ALU.mult, ALU.add)
                qNb.append(qn)
            _layernorm_batch(nc, p_sm, p_xh, qNb, g1b, be1b, qNb, eps_t,
                             affine1)
            for j in range(JB):
                qi = qb * JB + j
                qc_t = p_cst.tile([P, D], BF16, tag="cst", name=f"qc{qi}")
                nc.vector.tensor_copy(out=qc_t, in_=qNb[j])
                nc.sync.dma_start(out=scr_q.ap()[qi * P:(qi + 1) * P, :],
                                  in_=qc_t)
            qTb = []
            for k in range(DT):
                t = p_qb.tile([P, QB], BF16, tag="qTb", bufs=8,
                              name=f"qTb{qb}_{k}")
                nc.sync.dma_start(out=t,
                                  in_=scr_q.ap()[qs, k * P:(k + 1) * P],
                                  transpose=True)
                qTb.append(t)
            state[qb] = (qNb, qTb)

        def emit_ffn(qb):
            qNb, qTb = state.pop(qb)
            ff1 = []
            for m in range(HT):
                mcol = slice(m * P, (m + 1) * P)
                ps = ps_mm.tile([P, QB], F32, tag="mm", name=f"psf{qb}_{m}")
                for k in range(DT):
                    nc.tensor.matmul(ps, w1_b[k][:, mcol], qTb[k],
                                     start=(k == 0), stop=(k == DT - 1))
                ft = p_qb.tile([P, QB], BF16, tag="ff1", bufs=8,
                               name=f"ff1_{qb}_{m}")
                nc.scalar.activation(ft, ps, gelu_func, bias=b1_sb[m])
                ff1.append(ft)
            r2s = []
            for j in range(JB):
                qi = qb * JB + j
                ps = ps_mm.tile([P, D], F32, tag="mm", name=f"pso{qi}")
                for k in range(HT):
                    nc.tensor.matmul(ps, ff1[k][:, j * P:(j + 1) * P],
                                     w2_b[k],
                                     start=(k == 0), stop=(k == HT - 1))
                r2 = p_qb.tile([P, D], F32, tag="r2", bufs=4, name=f"r2_{qi}")
                nc.vector.tensor_add(r2, ps, qNb[j])
                if bias2:
                    nc.vector.tensor_add(r2, r2, b2b)
                r2s.append(r2)
            outs = [p_qb.tile([P, D], F32, tag="ot", bufs=4,
                              name=f"ot{qb}_{j}") for j in range(JB)]
            _layernorm_batch(nc, p_sm, p_xh, r2s, g2b, be2b, outs, eps_t,
                             affine2)
            for j in range(JB):
                qi = qb * JB + j
                nc.scalar.dma_start(out=dr["out"].ap()[qi * P:(qi + 1) * P, :],
                                    in_=outs[j])

        emit_scores(0)
        for qb in range(NQB):
            emit_attn_ln1_qt(qb)
            if qb + 1 < NQB:
                emit_scores(qb + 1)
            emit_ffn(qb)


def _layernorm_batch(nc, p_sm, p_xh, xs, gb, bb, outs, eps_t, affine):
    """outs[i] = LN(xs[i]) * gb + bb, batched to keep ACT functions grouped."""
    n = len(xs)
    mvs, sds, rstds, nmrs, xhs = [], [], [], [], []
    for x in xs:
        st = p_sm.tile([P, nc.vector.BN_STATS_DIM], F32, tag="bnst")
        nc.vector.bn_stats(st, x)
        mv = p_sm.tile([P, nc.vector.BN_AGGR_DIM], F32, tag="bnmv")
        nc.vector.bn_aggr(mv, st)
        mvs.append(mv)
    for i in range(n):
        sd = p_sm.tile([P, 1], F32, tag="sd")
        nc.scalar.activation(sd, mvs[i][:, 1:2], AF.Sqrt, bias=eps_t)
        sds.append(sd)
    for i in range(n):
        rstd = p_sm.tile([P, 1], F32, tag="rstd")
        nc.vector.reciprocal(rstd, sds[i])
        rstds.append(rstd)
    for i in range(n):
        nmr = p_sm.tile([P, 1], F32, tag="nmr")
        nc.vector.scalar_tensor_tensor(nmr, mvs[i][:, 0:1], -1.0, rstds[i],
                                       ALU.mult, ALU.mult)
        nmrs.append(nmr)
    for i in range(n):
        if affine:
            xh = p_xh.tile([P, D], F32, tag="xh")
            nc.vector.tensor_scalar(xh, xs[i], rstds[i], nmrs[i],
                                    ALU.mult, ALU.add)
            xhs.append(xh)
        else:
            nc.vector.tensor_scalar(outs[i], xs[i], rstds[i], nmrs[i],
                                    ALU.mult, ALU.add)
    if affine:
        for i in range(n):
            nc.vector.tensor_mul(xhs[i], xhs[i], gb)
        for i in range(n):
            nc.vector.tensor_add(outs[i], xhs[i], bb)


_IN_MAP = {
    "Noise_x": "nx", "X": "x", "cond": "cond",
    "W_gate": "wg", "b_gate": "bg", "W1": "w1", "b1": "b1",
    "W2": "w2", "b2": "b2", "g1": "g1", "be1": "be1",
    "g2": "g2", "be2": "be2",
}


def _run(inputs, trace=False):
    affine1 = not (np.all(np.asarray(inputs["g1"]) == 1.0)
                   and np.all(np.asarray(inputs["be1"]) == 0.0))
    affine2 = not (np.all(np.asarray(inputs["g2"]) == 1.0)
                   and np.all(np.asarray(inputs["be2"]) == 0.0))
    bias2 = not np.all(np.asarray(inputs["b2"]) == 0.0)
    key = ("nc", affine1, affine2, bias2)
    if key not in _cache:
        _cache[key] = _build(affine1=affine1, affine2=affine2, bias2=bias2)
    nc = _cache[key]

    in_maps = []
    for c in range(N_CORES):
        m = {}
        for src, dst in _IN_MAP.items():
            a = np.ascontiguousarray(np.asarray(inputs[src], dtype=np.float32))
            m[dst] = a[c] if a.ndim == 3 else a
        in_maps.append(m)
    res = run_bass_kernel_spmd(nc, in_maps, list(range(N_CORES)), trace=trace)
    out = np.stack([res.results[c]["out"] for c in range(N_CORES)], axis=0)
    return out, res


def kernel(**inputs) -> np.ndarray:
    out, _ = _run(inputs, trace=False)
    return out



# revision 18
# speedup vs baseline: 1.0356x; 1.0356x over previous
"""Trainium2 Bass kernel for nn_DenoisingConditionDecoder.

Per-core computation (data-parallel over batch, 1 batch element per core):
  gate  = sigmoid([nx, cond] @ W_gate + b_gate)
  fused = gate*nx + (1-gate)*cond
  attn  = softmax(fused @ X^T / sqrt(D)) @ X
  q     = LN(fused + attn) * g1 + be1
  ff    = gelu(q @ W1 + b1) @ W2 + b2
  out   = LN(q + ff) * g2 + be2

Layout strategy: activations feeding matmul contractions live transposed
([feat, seq], "T" layout) so the contraction dim sits on partitions;
softmax-normalize / layernorm / residuals run in natural [seq, feat]
layout.  Layout changes ride the DMA engines: bf16 copies are written to
DRAM scratch (SWDGE) and read back with the HWDGE xbar transpose on the
ACT ring, keeping PE/ACT/DVE compute free of transpose traffic and the
sync ring free of xbar-mode switches.

Attention uses a scores-transposed formulation: scoresT[k, q] tiles come
out of the PE with k on partitions, exp runs on ACT into fp8e4 tiles
which are directly the lhsT (stationary) operand of attn_out = attn @ X.
The whole attention block (scores, rowsum, attn) runs in fp8e4 with
DoubleRow perf mode (K=256 per instruction, 0.5 cyc/row): quantization
noise in the softmax weights and in X averages out over the ~1k-token
effective attention span, verified end-to-end <1e-3 extra error.  The
gate and FFN matmuls stay bf16 (fp8 there blows the 2e-2 budget).
Softmax row sums accumulate as ones-lhsT [1, QB] DoubleRow matmuls after
the scores chains; a tiny PE transpose gives per-partition reciprocals.
FFN2 uses the same trick: lhsT = ff1T tiles, rhs = W2 rows, producing
ff2 directly in natural layout for the residual.

Schedule: gate/fusion for all blocks runs first (retiring the condT/nxmcT
buffers, whose slots are then recycled as double-buffered exp blocks);
then a software pipeline over 512-query blocks emits FFN(qb-1) between
scores(qb) and attn(qb) so TensorE never waits on the LN1 -> q^T DMA
round-trip or the exp ACT latency.
"""

import math
import numpy as np

import concourse.bass as bass
import concourse.tile as tile
from concourse import bacc, mybir
from concourse.bass_utils import run_bass_kernel_spmd
from concourse.masks import make_identity

B, S, D = 8, 2048, 512
H = 2 * D
P = 128
NT = S // P   # 16 seq tiles
DT = D // P   # 4 feature tiles
HT = H // P   # 8 hidden tiles
QB = 512      # q-block
NQB = S // QB # 4
JB = QB // P  # 4 q-subtiles per block
LN_EPS = 1e-5
SCALE = 1.0 / math.sqrt(D)

F32 = mybir.dt.float32
BF16 = mybir.dt.bfloat16
FP8 = mybir.dt.float8e4
AF = mybir.ActivationFunctionType
ALU = mybir.AluOpType
DR = mybir.MatmulPerfMode.DoubleRow
NC = S // (2 * P)  # 8 row-pair chunks (DoubleRow k-tiles)

N_CORES = 8

_cache = {}


def _build(gelu_func=None, affine1=True, affine2=True, bias2=True):
    gelu_func = AF.Gelu if gelu_func is None else gelu_func
    nc = bacc.Bacc("TRN2", target_bir_lowering=False, debug=False,
                   num_devices=N_CORES)

    dr = {}
    for nm, shp in [("nx", [S, D]), ("x", [S, D]), ("cond", [S, D]),
                    ("wg", [H, D]), ("bg", [D]), ("w1", [D, H]), ("b1", [H]),
                    ("w2", [H, D]), ("b2", [D]), ("g1", [D]), ("be1", [D]),
                    ("g2", [D]), ("be2", [D])]:
        dr[nm] = nc.dram_tensor(nm, shp, F32, kind="ExternalInput")
    dr["out"] = nc.dram_tensor("out", [S, D], F32, kind="ExternalOutput")

    with tile.TileContext(nc) as tc:
        _body(nc, tc, dr, gelu_func, affine1, affine2, bias2)
    nc.compile()
    return nc


def _body(nc, tc, dr, gelu_func, affine1, affine2, bias2):
    from contextlib import ExitStack

    # DRAM scratch for DMA-based transposes (bf16)
    scr_x = nc.dram_tensor("scr_x", [S, D], BF16)
    scr_fT = nc.dram_tensor("scr_fT", [D, S], BF16)
    scr_q = nc.dram_tensor("scr_q", [S, D], BF16)

    ctx = ExitStack()
    with ctx:
        # ---------------- pools ----------------
        const = ctx.enter_context(tc.tile_pool(name="const", bufs=1))
        p_stg = ctx.enter_context(tc.tile_pool(name="stg", bufs=4))
        p_w = ctx.enter_context(tc.tile_pool(name="w", bufs=1))
        p_big = ctx.enter_context(tc.tile_pool(name="big", bufs=4))
        p_seq = ctx.enter_context(tc.tile_pool(name="seq", bufs=16))
        p_qb = ctx.enter_context(tc.tile_pool(name="qbp", bufs=8))
        p_cst = ctx.enter_context(tc.tile_pool(name="cst", bufs=6))
        p_sm = ctx.enter_context(tc.tile_pool(name="sm", bufs=8))
        p_xh = ctx.enter_context(tc.tile_pool(name="xh", bufs=4))
        p_f8 = ctx.enter_context(tc.tile_pool(name="f8", bufs=2))
        p_exp = ctx.enter_context(tc.tile_pool(name="exp", bufs=8))
        p_xt = ctx.enter_context(tc.tile_pool(name="xt", bufs=2))

        ps_mm = ctx.enter_context(tc.tile_pool(name="psmm", bufs=4,
                                               space="PSUM"))
        ps_tr = ctx.enter_context(tc.tile_pool(name="pstr", bufs=4,
                                               space="PSUM"))

        # ---------------- constants ----------------
        ident_f = const.tile([P, P], F32, tag="idf")
        make_identity(nc, ident_f)
        ident_b = const.tile([P, P], BF16, tag="idb")
        make_identity(nc, ident_b)
        # all-ones dual-fp8 lhsT: dual ldweights requires full M=128, so the
        # rowsum matmul broadcasts the k-sum to every PSUM partition.
        ones8 = const.tile([P, 2, P], FP8, tag="ones")
        nc.vector.memset(ones8, 1.0)
        eps_t = const.tile([P, 1], F32, tag="eps")
        nc.vector.memset(eps_t, LN_EPS)

        def bcast_vec(dram, tag):
            t = const.tile([P, D], F32, tag=tag)
            a = dram.ap()
            src = bass.AP(tensor=a.tensor, offset=a.offset,
                          ap=[[0, P]] + list(a.ap))
            nc.sync.dma_start(out=t, in_=src)
            return t

        g1b = bcast_vec(dr["g1"], "g1") if affine1 else None
        be1b = bcast_vec(dr["be1"], "be1") if affine1 else None
        g2b = bcast_vec(dr["g2"], "g2") if affine2 else None
        be2b = bcast_vec(dr["be2"], "be2") if affine2 else None
        b2b = bcast_vec(dr["b2"], "b2v") if bias2 else None

        def part_vec(dram, n, tag):
            ts = []
            for m in range(n):
                t = p_sm.tile([P, 1], F32, tag=tag, bufs=n, name=f"{tag}{m}")
                nc.sync.dma_start(
                    out=t, in_=dram.ap()[m * P:(m + 1) * P].unsqueeze(1))
                ts.append(t)
            return ts

        bg_sb = part_vec(dr["bg"], DT, "bg")
        b1_sb = part_vec(dr["b1"], HT, "b1")

        # ---------------- weights (cast f32 -> bf16 during DMA) ----------
        def load_w(dram, n, cols, tag, pref):
            ts = []
            for k in range(n):
                t = p_w.tile([P, cols], BF16, tag=tag, bufs=n,
                             name=f"{pref}{k}")
                nc.gpsimd.dma_start(out=t, in_=dram.ap()[k * P:(k + 1) * P, :])
                ts.append(t)
            return ts

        wg_b = load_w(dr["wg"], HT, D, "wg", "wg")
        # gate uses combined=[nx,cond]; rewrite with nxmc=(nx-cond):
        #   logits = nxmc @ Wg_top + cond @ (Wg_top + Wg_bot)
        for k in range(DT):
            nc.vector.tensor_add(wg_b[k + DT], wg_b[k + DT], wg_b[k])

        # ------- stage 1: load (batched), cast, scratch, transpose-load ---
        XN8 = [p_seq.tile([P, 2, D], FP8, tag="XN8", bufs=NC,
                          name=f"XN8_{c}") for c in range(NC)]
        condT = [p_big.tile([P, S], BF16, tag="condT", bufs=DT,
                            name=f"condT{j}") for j in range(DT)]
        nxmcT = [p_big.tile([P, S], BF16, tag="nxmcT", bufs=DT,
                            name=f"nxmcT{j}") for j in range(DT)]
        # XT8: [P, kt, pair, P] so each lhsT slice [:, kt] is contiguous
        # (dual-fp8 LDWEIGHTS requires contiguous stationary operands).
        XT8 = [p_f8.tile([P, NT, 2, P], FP8, tag="XT8", bufs=2,
                         name=f"XT8_{p}") for p in range(2)]
        fT8 = [p_f8.tile([P, 2, S], FP8, tag="fT8", bufs=2,
                         name=f"fT8_{p}") for p in range(2)]

        for qc in range(NQB):
            for a in range(2):  # two 256-row chunks per q-block
                c = qc * 2 + a
                rows = slice(c * 2 * P, (c + 1) * 2 * P)
                nx_s = p_stg.tile([P, 2, D], F32, tag="stg", name=f"nxs{qc}_{a}")
                nc.sync.dma_start(
                    out=nx_s, in_=dr["nx"].ap()[rows, :].rearrange(
                        "(t p) d -> p t d", p=P))
                cond_s = p_stg.tile([P, 2, D], F32, tag="stg",
                                    name=f"cds{qc}_{a}")
                nc.sync.dma_start(
                    out=cond_s, in_=dr["cond"].ap()[rows, :].rearrange(
                        "(t p) d -> p t d", p=P))
                x_s = p_stg.tile([P, 2, D], F32, tag="stg", name=f"xs{qc}_{a}")
                nc.sync.dma_start(
                    out=x_s, in_=dr["x"].ap()[rows, :].rearrange(
                        "(t p) d -> p t d", p=P))
                nc.vector.tensor_copy(out=XN8[c], in_=x_s)
                xb = p_cst.tile([P, 2, D], BF16, tag="cstw", bufs=2,
                                name=f"xb{c}")
                nc.vector.tensor_copy(out=xb, in_=x_s)
                nc.sync.dma_start(
                    out=scr_x.ap()[rows, :].rearrange("(t p) d -> p t d", p=P),
                    in_=xb)
                for h in range(2):
                    i = c * 2 + h
                    row = slice(i * P, (i + 1) * P)
                    cb = p_cst.tile([P, D], BF16, tag="cst", name=f"cb{i}")
                    nc.vector.tensor_copy(out=cb, in_=cond_s[:, h, :])
                    sb = p_cst.tile([P, D], BF16, tag="cst", name=f"sb{i}")
                    nc.vector.tensor_sub(sb, nx_s[:, h, :], cond_s[:, h, :])
                    for j in range(DT):
                        col = slice(j * P, (j + 1) * P)
                        ptc = ps_tr.tile([P, P], BF16, tag="tr",
                                         name=f"ptc{i}_{j}")
                        nc.tensor.transpose(ptc, cb[:, col], ident_b)
                        nc.any.tensor_copy(out=condT[j][:, row], in_=ptc)
                        ptn = ps_tr.tile([P, P], BF16, tag="tr",
                                         name=f"ptn{i}_{j}")
                        nc.tensor.transpose(ptn, sb[:, col], ident_b)
                        nc.any.tensor_copy(out=nxmcT[j][:, row], in_=ptn)

        for j in range(DT):
            col = slice(j * P, (j + 1) * P)
            for h in range(2):
                rows = slice(h * (S // 2), (h + 1) * (S // 2))
                xtt = p_xt.tile([P, S // 2], BF16, tag="xtt",
                                name=f"xtt{j}_{h}")
                nc.sync.dma_start(out=xtt, in_=scr_x.ap()[rows, col],
                                  transpose=True)
                nc.any.tensor_copy(
                    out=XT8[j // 2][:, h * 8:(h + 1) * 8, j % 2, :],
                    in_=xtt[:, :].rearrange("p (a c) -> p a c", a=8))

        # ------- stage 2: gate matmul + fusion for ALL blocks -------------
        for qc in range(NQB):
            qs = slice(qc * QB, (qc + 1) * QB)
            for m in range(DT):
                mcol = slice(m * P, (m + 1) * P)
                ps = ps_mm.tile([P, QB], F32, tag="mm", name=f"psg{qc}_{m}")
                for k in range(HT):
                    src = nxmcT[k] if k < DT else condT[k - DT]
                    nc.tensor.matmul(ps, wg_b[k][:, mcol], src[:, qs],
                                     start=(k == 0), stop=(k == HT - 1))
                gt = p_cst.tile([P, QB], BF16, tag="cst", name=f"gt{qc}_{m}")
                nc.scalar.activation(gt, ps, AF.Sigmoid, bias=bg_sb[m])
                # fusedT = cond + gate * (nx - cond)
                nc.vector.tensor_mul(gt, gt, nxmcT[m][:, qs])
                ft_s = p_cst.tile([P, QB], BF16, tag="cst", name=f"fts{qc}_{m}")
                nc.vector.tensor_add(ft_s, gt, condT[m][:, qs])
                nc.sync.dma_start(out=scr_fT.ap()[mcol, qs], in_=ft_s)
                nc.any.tensor_copy(out=fT8[m // 2][:, m % 2, qs], in_=ft_s)
        w1_b = load_w(dr["w1"], DT, H, "w1", "w1")
        w2_b = load_w(dr["w2"], HT, D, "w2", "w2")
        fN = [p_seq.tile([P, D], BF16, tag="fN", bufs=NT, name=f"fN{qi}")
              for qi in range(NT)]
        for qi in range(NT):
            nc.sync.dma_start(out=fN[qi],
                                in_=scr_fT.ap()[:, qi * P:(qi + 1) * P],
                                transpose=True)

        # ------- per-q-block software pipeline ----------------------------
        state = {}

        def emit_scores(qb):
            qs = slice(qb * QB, (qb + 1) * QB)
            # eblk layout [P, j, pair, P]: attn lhsT slices [:, j] contiguous
            eblk = [p_exp.tile([P, JB, 2, P], FP8, tag="eblk", bufs=8,
                               name=f"eblk{qb}_{b}") for b in range(NC)]
            for kt in range(NT):
                ps = ps_mm.tile([P, JB, P], F32, tag="mm",
                                name=f"pss{qb}_{kt}")
                for p in range(2):
                    nc.tensor.matmul(ps, XT8[p][:, kt, :, :],
                                     fT8[p][:, :, qs], perf_mode=DR,
                                     start=(p == 0), stop=(p == 1))
                esl = eblk[kt // 2][:, :, kt % 2, :]
                nc.scalar.activation(esl, ps, AF.Exp, scale=SCALE)
            prs = ps_mm.tile([P, QB], F32, tag="mm", name=f"prs{qb}")
            for c in range(NC):
                rs_rhs = eblk[c][:, :, :, :].rearrange("p j two c -> p two j c")
                nc.tensor.matmul(prs, ones8, rs_rhs, perf_mode=DR,
                                 start=(c == 0), stop=(c == NC - 1))
            rs_sb = p_sm.tile([1, QB], F32, tag="rs", bufs=4,
                              name=f"rssb{qb}")
            nc.vector.tensor_copy(out=rs_sb, in_=prs[0:1, :])
            state[qb] = (eblk, rs_sb)

        def emit_attn_ln1_qt(qb):
            eblk, rs_sb = state[qb]
            qs = slice(qb * QB, (qb + 1) * QB)
            qNb = []
            for j in range(JB):
                qi = qb * JB + j
                pa = ps_mm.tile([P, D], F32, tag="mm", name=f"psa{qi}")
                for c in range(NC):
                    lhs = eblk[c][:, j, :, :]
                    nc.tensor.matmul(pa, lhs, XN8[c], perf_mode=DR,
                                     start=(c == 0), stop=(c == NC - 1))
                prt = ps_tr.tile([P, 1], F32, tag="tr", name=f"prt{qi}")
                nc.tensor.transpose(prt, rs_sb[0:1, j * P:(j + 1) * P],
                                    ident_f[0:1, 0:1])
                rec = p_sm.tile([P, 1], F32, tag="rec", name=f"rec{qi}")
                nc.vector.reciprocal(rec, prt)
                qn = p_qb.tile([P, D], F32, tag="qN", bufs=8, name=f"qN{qi}")
                # r1 = attn_out/rowsum + fused   (LN1 runs in place)
                nc.vector.scalar_tensor_tensor(
                    qn, pa, rec, fN[qi], ALU.mult, ALU.add)
                qNb.append(qn)
            _layernorm_batch(nc, p_sm, p_xh, qNb, g1b, be1b, qNb, eps_t,
                             affine1)
            for j in range(JB):
                qi = qb * JB + j
                qc_t = p_cst.tile([P, D], BF16, tag="cst", name=f"qc{qi}")
                nc.vector.tensor_copy(out=qc_t, in_=qNb[j])
                nc.sync.dma_start(out=scr_q.ap()[qi * P:(qi + 1) * P, :],
                                  in_=qc_t)
            qTb = []
            for k in range(DT):
                t = p_qb.tile([P, QB], BF16, tag="qTb", bufs=8,
                              name=f"qTb{qb}_{k}")
                nc.sync.dma_start(out=t,
                                  in_=scr_q.ap()[qs, k * P:(k + 1) * P],
                                  transpose=True)
                qTb.append(t)
            state[qb] = (qNb, qTb)

        def emit_ffn(qb):
            qNb, qTb = state.pop(qb)
            ff1 = []
            for m in range(HT):
                mcol = slice(m * P, (m + 1) * P)
                ps = ps_mm.tile([P, QB], F32, tag="mm", name=f"psf{qb}_{m}")
                for k in range(DT):
                    nc.tensor.matmul(ps, w1_b[k][:, mcol], qTb[k],
                                     start=(k == 0), stop=(k == DT - 1))
                ft = p_qb.tile([P, QB], BF16, tag="ff1", bufs=8,
                               name=f"ff1_{qb}_{m}")
                nc.scalar.activation(ft, ps, gelu_func, bias=b1_sb[m])
                ff1.append(ft)
            r2s = []
            for j in range(JB):
                qi = qb * JB + j
                ps = ps_mm.tile([P, D], F32, tag="mm", name=f"pso{qi}")
                for k in range(HT):
                    nc.tensor.matmul(ps, ff1[k][:, j * P:(j + 1) * P],
                                     w2_b[k],
                                     start=(k == 0), stop=(k == HT - 1))
                r2 = p_qb.tile([P, D], F32, tag="r2", bufs=4, name=f"r2_{qi}")
                nc.vector.tensor_add(r2, ps, qNb[j])
                if bias2:
                    nc.vector.tensor_add(r2, r2, b2b)
                r2s.append(r2)
            outs = [p_qb.tile([P, D], F32, tag="ot", bufs=4,
                              name=f"ot{qb}_{j}") for j in range(JB)]
            _layernorm_batch(nc, p_sm, p_xh, r2s, g2b, be2b, outs, eps_t,
                             affine2)
            for j in range(JB):
                qi = qb * JB + j
                nc.scalar.dma_start(out=dr["out"].ap()[qi * P:(qi + 1) * P, :],
                                    in_=outs[j])

        emit_scores(0)
        for qb in range(NQB):
            emit_attn_ln1_qt(qb)
            if qb + 1 < NQB:
                emit_scores(qb + 1)
            emit_ffn(qb)


def _layernorm_batch(nc, p_sm, p_xh, xs, gb, bb, outs, eps_t, affine):
    """outs[i] = LN(xs[i]) * gb + bb, batched to keep ACT functions grouped."""
    n = len(xs)
    mvs, sds, rstds, nmrs, xhs = [], [], [], [], []
    for x in xs:
        st = p_sm.tile([P, nc.vector.BN_STATS_DIM], F32, tag="bnst")
        nc.vector.bn_stats(st, x)
        mv = p_sm.tile([P, nc.vector.BN_AGGR_DIM], F32, tag="bnmv")
        nc.vector.bn_aggr(mv, st)
        mvs.append(mv)
    for i in range(n):
        sd = p_sm.tile([P, 1], F32, tag="sd")
        nc.scalar.activation(sd, mvs[i][:, 1:2], AF.Sqrt, bias=eps_t)
        sds.append(sd)
    for i in range(n):
        rstd = p_sm.tile([P, 1], F32, tag="rstd")
        nc.vector.reciprocal(rstd, sds[i])
        rstds.append(rstd)
    for i in range(n):
        nmr = p_sm.tile([P, 1], F32, tag="nmr")
        nc.vector.scalar_tensor_tensor(nmr, mvs[i][:, 0:1], -1.0, rstds[i],
                                       ALU.mult, ALU.mult)
        nmrs.append(nmr)
    for i in range(n):
        if affine:
            xh = p_xh.tile([P, D], F32, tag="xh")
            nc.vector.tensor_scalar(xh, xs[i], rstds[i], nmrs[i],
                                    ALU.mult, ALU.add)
            xhs.append(xh)
        else:
            nc.vector.tensor_scalar(outs[i], xs[i], rstds[i], nmrs[i],
                                    ALU.mult, ALU.add)
    if affine:
        for i in range(n):
            nc.vector.tensor_mul(xhs[i], xhs[i], gb)
        for i in range(n):
            nc.vector.tensor_add(outs[i], xhs[i], bb)


_IN_MAP = {
    "Noise_x": "nx", "X": "x", "cond": "cond",
    "W_gate": "wg", "b_gate": "bg", "W1": "w1", "b1": "b1",
    "W2": "w2", "b2": "b2", "g1": "g1", "be1": "be1",
    "g2": "g2", "be2": "be2",
}


def _run(inputs, trace=False):
    affine1 = not (np.all(np.asarray(inputs["g1"]) == 1.0)
                   and np.all(np.asarray(inputs["be1"]) == 0.0))
    affine2 = not (np.all(np.asarray(inputs["g2"]) == 1.0)
                   and np.all(np.asarray(inputs["be2"]) == 0.0))
    bias2 = not np.all(np.asarray(inputs["b2"]) == 0.0)
    key = ("nc", affine1, affine2, bias2)
    if key not in _cache:
        _cache[key] = _build(affine1=affine1, affine2=affine2, bias2=bias2)
    nc = _cache[key]

    in_maps = []
    for c in range(N_CORES):
        m = {}
        for src, dst in _IN_MAP.items():
            a = np.ascontiguousarray(np.asarray(inputs[src], dtype=np.float32))
            m[dst] = a[c] if a.ndim == 3 else a
        in_maps.append(m)
    res = run_bass_kernel_spmd(nc, in_maps, list(range(N_CORES)), trace=trace)
    out = np.stack([res.results[c]["out"] for c in range(N_CORES)], axis=0)
    return out, res


def kernel(**inputs) -> np.ndarray:
    out, _ = _run(inputs, trace=False)
    return out

